# revision 1
# baseline (speedup 1.0000x reference)
"""Binarized-weight 3-layer MLP on 8 Trainium2 NeuronCores (Bass/Tile).

Reference computation (per-tensor scalar binarization):
    h1 = relu(x @ (sign(w1)*mean|w1|).T + b1)
    h2 = relu(h1 @ (sign(w2)*mean|w2|).T + b2)
    out = sigmoid(h2 @ (sign(w3)*mean|w3|).T + b3)

Strategy: data-parallel over batch (8192 rows -> 1024 rows/core), weights
replicated.  Per core everything is kept feature-major ("transposed"):
activations live in SBUF as [feature_partition, batch_free] so layer l's
output is directly layer l+1's matmul moving operand.  Weights are
pre-tiled on the host to [strip, k_partition, k_tile*feat] so each strip
DMA is a single transfer with 16KB contiguous per partition.

Binarization happens on device: ACT engine computes sign(w) in bf16
(exact +-1), DVE computes per-strip sum|w| partials, and a ones-matmul
does the final cross-partition sum + broadcast.  alpha is folded into the
layer-boundary activation: relu(alpha*z + b) is one ACT op per tile.
Matmuls run in bf16 (sign weights exact; x rounded to bf16) with fp32
PSUM accumulation.
"""

import numpy as np
from contextlib import ExitStack

import concourse.bass as bass
import concourse.tile as tile
from concourse import bacc, mybir
from concourse.bass_utils import run_bass_kernel_spmd

N_CORES = 8
F32 = mybir.dt.float32
BF16 = mybir.dt.bfloat16
AF = mybir.ActivationFunctionType
AX = mybir.AxisListType
ALU = mybir.AluOpType

# Full-problem dims (hardcoded; harness calls kernel() with these shapes)
IN_SIZE, HIDDEN, OUT_SIZE, BATCH = 4096, 4096, 1024, 8192


def build_mlp(B, IN, H, OUT, n_cores=N_CORES, repeats=1,
              skip_wdma=False, skip_sign=False, skip_reduce=False,
              skip_mm=False, nb=None, binz_mode="act_dve"):
    """Build the single-core SPMD program for a per-core batch of B.

    repeats>1 wraps the whole body in a hardware For_i loop — used only for
    amortized timing (slope between two repeat counts cancels dispatch
    overhead).  The skip_* flags build timing probes (outputs garbage):
    skip_wdma drops weight DMAs (+sign+reduce, const weights), skip_sign
    feeds the PE a bitcast view of the f32 strip, skip_reduce uses a
    constant alpha, skip_mm drops the matmuls+evictions."""
    NB = nb if nb is not None else min(512, B)  # matmul free dim
    NBC = B // NB             # batch chunks per strip
    assert B % NB == 0
    KT1, FT1 = IN // 128, H // 128      # layer 1: k-tiles, feature strips
    KT2, FT2 = H // 128, H // 128
    KT3, FT3 = H // 128, OUT // 128

    nc = bacc.Bacc("TRN2", target_bir_lowering=False, debug=False,
                   enable_asserts=True, num_devices=n_cores)

    xT = nc.dram_tensor("xT", [IN, B], F32, kind="ExternalInput").ap()
    w1s = nc.dram_tensor("w1s", [FT1, 128, IN], F32, kind="ExternalInput").ap()
    w2s = nc.dram_tensor("w2s", [FT2, 128, H], F32, kind="ExternalInput").ap()
    w3s = nc.dram_tensor("w3s", [FT3, 128, H], F32, kind="ExternalInput").ap()
    b1t = nc.dram_tensor("b1t", [128, FT1], F32, kind="ExternalInput").ap()
    b2t = nc.dram_tensor("b2t", [128, FT2], F32, kind="ExternalInput").ap()
    b3t = nc.dram_tensor("b3t", [128, FT3], F32, kind="ExternalInput").ap()
    out = nc.dram_tensor("out", [OUT, B], F32, kind="ExternalOutput").ap()

    with tile.TileContext(nc) as tc, ExitStack() as ctx:
        persist = ctx.enter_context(tc.tile_pool(name="persist", bufs=1))
        wpool = ctx.enter_context(tc.tile_pool(name="wf32", bufs=2))
        spool = ctx.enter_context(tc.tile_pool(name="wsgn", bufs=2))
        stage = ctx.enter_context(tc.tile_pool(name="stage", bufs=2))
        psum_bufs = 6 if NB <= 512 else 3
        psum = ctx.enter_context(tc.tile_pool(name="psum", bufs=psum_bufs, space="PSUM"))
        apsum = ctx.enter_context(tc.tile_pool(name="apsum", bufs=1, space="PSUM"))

        if repeats > 1:
            ctx.enter_context(tc.For_i(0, repeats, 1))

        # Ping-pong activation buffers, feature-major bf16.
        colsA = max(KT1, FT2, KT3) * B
        colsB = max(FT1, KT2, FT3) * B
        bufA = persist.tile([128, colsA], BF16, tag="bufA")
        bufB = persist.tile([128, colsB], BF16, tag="bufB")

        ones = persist.tile([128, 128], F32, tag="ones")
        nc.vector.memset(ones[:], 1.0)

        btiles = []
        for li, (bt_d, FT) in enumerate([(b1t, FT1), (b2t, FT2), (b3t, FT3)]):
            t = persist.tile([128, FT], F32, tag=f"bias{li}")
            nc.sync.dma_start(t[:], bt_d[:, :])
            btiles.append(t)

        # Load x with casting DMAs (SWDGE queue, separate from the HWDGE
        # weight-strip queue): f32 DRAM -> bf16 SBUF directly.
        for kt in range(KT1):
            nc.gpsimd.dma_start(bufA[:, kt * B:(kt + 1) * B],
                                xT[kt * 128:(kt + 1) * 128, :])

        wconst = None
        if skip_wdma:
            wconst = persist.tile([128, max(IN, H)], BF16, tag="wconst")
            nc.vector.memset(wconst[:], 1.0)

        def layer(li, wdram, CT, FT, rhsbuf, zbuf):
            """Matmul layer: zbuf[:, ft*B+..] = sign(w_l) rows @ rhsbuf.
            Returns the alpha (mean|w|) broadcast tile [128,1] f32."""
            C = CT * 128
            partials = persist.tile([128, FT], F32, tag=f"partials{li}")
            for ft in range(FT):
                if skip_wdma:
                    ws = wconst
                else:
                    wf = wpool.tile([128, C], F32, tag="wf32")
                    nc.sync.dma_start(wf[:], wdram[ft, :, :])
                    if skip_sign:
                        ws = wf.bitcast(BF16)[:, :C]
                    else:
                        ws = spool.tile([128, C], BF16, tag="wsgn")
                        if binz_mode == "act2" and not skip_reduce:
                            # |w| sums via ACT accum_out; abs output is a
                            # dump (ws is rewritten by Sign right after).
                            nc.scalar.activation(
                                ws[:], wf[:], AF.Abs,
                                accum_out=partials[:, ft:ft + 1])
                        nc.scalar.activation(ws[:], wf[:], AF.Sign)
                    if skip_reduce or binz_mode == "act2":
                        pass
                    else:
                        nc.vector.tensor_reduce(
                            partials[:, ft:ft + 1], wf[:], axis=AX.X, op=ALU.add,
                            apply_absolute_value=True)
                if skip_mm:
                    continue
                for bc in range(NBC):
                    pt = psum.tile([128, NB], F32, tag="psum")
                    for ct in range(CT):
                        nc.tensor.matmul(
                            pt[:],
                            ws[:, ct * 128:(ct + 1) * 128],
                            rhsbuf[:, ct * B + bc * NB: ct * B + bc * NB + NB],
                            start=(ct == 0), stop=(ct == CT - 1))
                    nc.vector.tensor_copy(
                        zbuf[:, ft * B + bc * NB: ft * B + bc * NB + NB], pt[:])
            if skip_wdma or skip_reduce:
                alpha = persist.tile([128, 1], F32, tag=f"alpha{li}")
                nc.vector.memset(alpha[:], 0.0078)
                return alpha
            # alpha = mean(|w|): reduce partials, then ones-matmul for
            # cross-partition sum broadcast to all 128 partitions.
            rsum = persist.tile([128, 1], F32, tag=f"rsum{li}")
            nc.vector.tensor_reduce(rsum[:], partials[:, :], axis=AX.X, op=ALU.add)
            ap_ps = apsum.tile([128, 1], F32, tag="apsum")
            nc.tensor.matmul(ap_ps[:], ones[:], rsum[:], start=True, stop=True)
            alpha = persist.tile([128, 1], F32, tag=f"alpha{li}")
            nc.scalar.mul(alpha[:], ap_ps[:], 1.0 / (FT * 128 * C))
            return alpha

        def relu_boundary(buf, FT, bias_t, alpha):
            """h = relu(alpha*z + b) in place, tiles alternating ACT/DVE so
            the next layer's first strip isn't rate-limited by one engine."""
            for ft in range(FT):
                sl = buf[:, ft * B:(ft + 1) * B]
                if ft % 2 == 0:
                    nc.scalar.activation(sl, sl, AF.Relu,
                                         bias=bias_t[:, ft:ft + 1],
                                         scale=alpha[:, :])
                else:
                    nc.vector.tensor_scalar(
                        sl, sl, alpha[:, :], bias_t[:, ft:ft + 1],
                        ALU.mult, ALU.add)
                    nc.vector.tensor_scalar_max(sl, sl, 0.0)

        # Layer 1: rhs = bufA (x), z1 -> bufB
        a1 = layer(0, w1s, KT1, FT1, bufA, bufB)
        relu_boundary(bufB, FT1, btiles[0], a1)

        # Layer 2: rhs = bufB (h1), z2 -> bufA (x is dead)
        a2 = layer(1, w2s, KT2, FT2, bufB, bufA)
        relu_boundary(bufA, FT2, btiles[1], a2)

        # Layer 3: rhs = bufA (h2), z3 -> bufB (z1/h1 dead)
        a3 = layer(2, w3s, KT3, FT3, bufA, bufB)
        for ft in range(FT3):  # out = sigmoid(a3*z3 + b3) -> f32 -> DRAM
            og = stage.tile([128, B], F32, tag="stage")
            nc.scalar.activation(og[:], bufB[:, ft * B:(ft + 1) * B], AF.Sigmoid,
                                 bias=btiles[2][:, ft:ft + 1], scale=a3[:, :])
            nc.sync.dma_start(out[ft * 128:(ft + 1) * 128, :], og[:])

    nc.compile()
    return nc


def _tile_weights(w, C):
    """(F, C) row-major -> [FT, 128, C] with per-strip layout [cp, ct*ff]."""
    F = w.shape[0]
    FT, CT = F // 128, C // 128
    return np.ascontiguousarray(
        w.reshape(FT, 128, CT, 128).transpose(0, 3, 2, 1).reshape(FT, 128, C))


def _tile_bias(b):
    """(F,) -> [128, FT] with b_t[p, t] = b[t*128 + p]."""
    FT = b.shape[0] // 128
    return np.ascontiguousarray(b.reshape(FT, 128).T)


def prepare_inputs(x, w1, b1, w2, b2, w3, b3, n_cores=N_CORES):
    """Host-side shard + relayout. Returns in_maps for run_bass_kernel_spmd."""
    x = np.asarray(x, dtype=np.float32)
    shared = {
        "w1s": _tile_weights(np.asarray(w1, np.float32), IN_SIZE),
        "w2s": _tile_weights(np.asarray(w2, np.float32), HIDDEN),
        "w3s": _tile_weights(np.asarray(w3, np.float32), HIDDEN),
        "b1t": _tile_bias(np.asarray(b1, np.float32)),
        "b2t": _tile_bias(np.asarray(b2, np.float32)),
        "b3t": _tile_bias(np.asarray(b3, np.float32)),
    }
    Bc = x.shape[0] // n_cores
    in_maps = []
    for c in range(n_cores):
        m = dict(shared)
        m["xT"] = np.ascontiguousarray(x[c * Bc:(c + 1) * Bc].T)
        in_maps.append(m)
    return in_maps


_NC_CACHE = {}


def kernel(x, w1, b1, w2, b2, w3, b3):
    key = "full"
    if key not in _NC_CACHE:
        _NC_CACHE[key] = build_mlp(BATCH // N_CORES, IN_SIZE, HIDDEN, OUT_SIZE)
    nc = _NC_CACHE[key]
    in_maps = prepare_inputs(x, w1, b1, w2, b2, w3, b3)
    res = run_bass_kernel_spmd(nc, in_maps, core_ids=list(range(N_CORES)))
    # per-core out is [OUT, Bc] feature-major; transpose + concat over batch
    return np.concatenate([r["out"].T for r in res.results], axis=0)

